# revision 20
# baseline (speedup 1.0000x reference)
"""Trainium2 Bass kernel for nn_CliffordFourierHead (CGENN-style Clifford net).

Network (per reference): B=1024, IN=256, HID=512, OUT=128, Cl(3,0), 8 blades.
  fcgp1 -> MVSiLU -> channel-wise steerable GP -> MVSiLU -> fcgp2

Strategy (v3):
  - Pure batch data-parallelism over 8 NeuronCores (128 batch rows each).
  - Channels on partitions, batch on free dim; an activation is 8 blade
    planes packed into one [128, 8*128] SBUF tile per channel-tile.
  - Geometric products: ONE wide DVE op pair builds a mega product tile
    Q[i,k] = x_i * xr_k; the Cayley contraction is absorbed into TensorE
    matmuls that accumulate into PSUM (negative signs via negated weight
    copies).
  - PSUM regions packed 2 banks per output tile: bank A = blades 0..3,
    bank B = blades 4..7 -> 4 regions in flight -> cross-phase overlap.
  - Emission order interleaves each phase's early-kt matmuls under the
    previous phase's normalize/silu tail so the PE never starves.
  - All weights pre-laid-out host-side as [128, W] contiguous slabs ->
    single fat DMA descriptors; stage-3 linear weights get their own SBUF
    region up front so their DMA overlaps stage-1 compute.
  - fp16 on-chip compute, fp32 PSUM/params.

Self-contained: shapes and the Cl(3,0) Cayley table are derived inline.
"""

import contextlib
import math

import numpy as np

NCORES = 8
B, NIN, HID, NOUT = 1024, 256, 512, 128
BC = B // NCORES  # 128 batch rows per core
NB = 8
KT_IN, KT_HID = NIN // 128, HID // 128  # 2, 4
MT_IN, MT_HID, MT_OUT = NIN // 128, HID // 128, NOUT // 128  # 2, 4, 1
GRADE_SLICES = [(0, 1), (1, 4), (4, 7), (7, 8)]
GRADE_OF = [0, 1, 1, 1, 2, 2, 2, 3]
EPS = 1e-6
ISQ2 = 1.0 / math.sqrt(2.0)


def _build_cayley():
    masks = sorted(range(NB), key=lambda m: (bin(m).count("1"), m))
    pos = {m: i for i, m in enumerate(masks)}
    cay = np.zeros((NB, NB, NB), dtype=np.float32)
    for i, mi in enumerate(masks):
        for k, mk in enumerate(masks):
            a, s = mi >> 1, 0
            while a:
                s += bin(a & mk).count("1")
                a >>= 1
            cay[i, pos[mi ^ mk], k] = -1.0 if (s & 1) else 1.0
    triples = []
    for gi in range(4):
        for gj in range(4):
            for gk in range(4):
                (i0, i1), (j0, j1), (k0, k1) = (
                    GRADE_SLICES[gi], GRADE_SLICES[gj], GRADE_SLICES[gk])
                if np.any(cay[i0:i1, j0:j1, k0:k1] != 0):
                    triples.append((gi, gj, gk))
    return cay, triples


CAY, TRIPLES = _build_cayley()
NPATHS = len(TRIPLES)  # 20

# Per triple t: {j: [(i, k, sign), ...]}
TRIPLE_TERMS = []
for t, (gi, gj, gk) in enumerate(TRIPLES):
    (i0, i1), (k0, k1) = GRADE_SLICES[gi], GRADE_SLICES[gk]
    d = {}
    for i in range(i0, i1):
        for k in range(k0, k1):
            j = int(np.nonzero(CAY[i, :, k])[0][0])
            if GRADE_SLICES[gj][0] <= j < GRADE_SLICES[gj][1]:
                d.setdefault(j, []).append((i, k, float(CAY[i, j, k])))
    TRIPLE_TERMS.append(d)


def _build_term_sets():
    """Per triple: list of matmul term-sets (j0, L, plane0, plane_step, sign).

    A term-set is a run of consecutive output blades j0..j0+L-1, one product
    plane each, uniform Cayley sign, arithmetic plane offsets (plane = i*8+k
    in the mega product tile) -> one matmul with a strided rhs plane AP.
    """
    all_sets = []
    for t in range(NPATHS):
        terms = []
        for j, lst in TRIPLE_TERMS[t].items():
            for (i, k, s) in lst:
                terms.append((j, i * 8 + k, s))
        sets = []
        for sgn in (1.0, -1.0):
            pool = sorted(x for x in terms if x[2] == sgn)
            while pool:
                j0, o0, _ = pool.pop(0)
                run = [(j0, o0)]
                step = None
                while True:
                    pick = None
                    for c in pool:
                        if c[0] != run[-1][0] + 1:
                            continue
                        st = c[1] - run[-1][1]
                        if step is None or st == step:
                            pick, pstep = c, st
                            break
                    if pick is None:
                        break
                    step = pstep
                    pool.remove(pick)
                    run.append((pick[0], pick[1]))
                sets.append((run[0][0], len(run), run[0][1], step or 0, sgn))
        all_sets.append(sets)
    return all_sets


TERM_SETS = _build_term_sets()
NEG_TRIPLES = sorted({t for t in range(NPATHS)
                      if any(s[4] < 0 for s in TERM_SETS[t])})
NEG_SLOT = {t: n for n, t in enumerate(NEG_TRIPLES)}
NNEG = len(NEG_TRIPLES)
NEG_RUNS = []
_i = 0
while _i < NNEG:
    _j = _i
    while _j + 1 < NNEG and NEG_TRIPLES[_j + 1] == NEG_TRIPLES[_j] + 1:
        _j += 1
    NEG_RUNS.append((NEG_TRIPLES[_i], _j - _i + 1))
    _i = _j + 1

# GP term-sets grouped by output grade; bank 0 = grades {0,1}, 1 = {2,3}
GP_SETS_BY_GRADE = {g: [(t, s) for t in range(NPATHS)
                        if TRIPLES[t][1] == g
                        for s in TERM_SETS[t]]
                    for g in range(4)}
NSETS_BANK = {0: len(GP_SETS_BY_GRADE[0]) + len(GP_SETS_BY_GRADE[1]),
              1: len(GP_SETS_BY_GRADE[2]) + len(GP_SETS_BY_GRADE[3])}


# ----------------------------------------------------------------------------
# Host-side prep (all tensors laid out [128, W] contiguous)
# ----------------------------------------------------------------------------
def prep_in_maps(inputs):
    f16, f32 = np.float16, np.float32

    def lin_w(w, scale=1.0):
        # [m, n, 4] -> [128, kt*4*m]
        m, n, _ = np.asarray(w).shape
        wt = np.transpose(np.asarray(w, f32), (1, 2, 0))  # [n, 4, m]
        wt = wt.reshape(n // 128, 128, 4, m).transpose(1, 0, 2, 3)
        return np.ascontiguousarray(wt * scale).reshape(128, -1).astype(f16)

    def gp_w(w, scale):
        # [m, n, 20] -> pos [128, kt*20*m], neg [128, kt*12*m]
        m, n, _ = np.asarray(w).shape
        wt = np.transpose(np.asarray(w, f32), (2, 1, 0)) * scale  # [20, n, m]
        wt = wt.reshape(NPATHS, n // 128, 128, m).transpose(2, 1, 0, 3)
        pos = np.ascontiguousarray(wt).reshape(128, -1).astype(f16)
        neg = np.ascontiguousarray(-wt[:, :, NEG_TRIPLES, :]).reshape(
            128, -1).astype(f16)
        return pos, neg

    def sig(a):
        return 1.0 / (1.0 + np.exp(-np.asarray(a, f32)))

    x = np.asarray(inputs["x"], f32)

    c = {}
    c["lr1w"] = lin_w(inputs["lr1_w"])
    c["ll1w"] = lin_w(inputs["ll1_w"], ISQ2)
    c["lrgw"] = lin_w(inputs["lrg_w"])
    c["llgw"] = lin_w(inputs["llg_w"], ISQ2)
    c["lr2w"] = lin_w(inputs["lr2_w"])
    c["ll2w"] = lin_w(inputs["ll2_w"], ISQ2)
    c["w1w"], _ = gp_w(inputs["w1"], ISQ2)
    c["w2w"], c["w2n"] = gp_w(inputs["w2"], ISQ2)

    # channel-wise GP weights as diagonal matrices [128, ct*20*128] (+neg)
    wg = np.asarray(inputs["wg"], f32) * ISQ2  # [HID, 20]
    dwg = np.zeros((MT_HID, NPATHS, 128, 128), f32)
    idx = np.arange(128)
    for t in range(NPATHS):
        wv = wg[:, t].reshape(MT_HID, 128)
        for ct in range(MT_HID):
            dwg[ct, t, idx, idx] = wv[ct]
    dwgp = dwg.transpose(2, 0, 1, 3)  # [128, ct, 20, 128]
    c["dwg"] = np.ascontiguousarray(dwgp).reshape(128, -1).astype(f16)
    c["dwgn"] = np.ascontiguousarray(
        -dwgp[:, :, NEG_TRIPLES, :]).reshape(128, -1).astype(f16)

    cols = []   # list of [128, w] blocks; order must match device PARAM map

    def addp(arr):
        cols.append(np.asarray(arr, f32).reshape(128, -1))

    for nm, a, kt in (("n1", inputs["n1_a"], KT_IN),
                      ("ng", inputs["ng_a"], KT_HID),
                      ("n2", inputs["n2_a"], KT_HID)):
        sa = sig(a).reshape(kt, 128, 4)
        cb = (1.0 + EPS) - sa
        for u in range(kt):
            addp(sa[u])
            addp(cb[u])
    aa = np.asarray(inputs["act_a"], f32).reshape(MT_HID, 128, 4)
    ab = np.asarray(inputs["act_b"], f32).reshape(MT_HID, 128, 4)
    for u in range(MT_HID):
        addp(aa[u])
        addp(ab[u])
    addp((np.asarray(inputs["ll1_b"], f32) * ISQ2).reshape(MT_HID, 128).T)
    addp((np.asarray(inputs["llg_b"], f32) * ISQ2).reshape(MT_HID, 128).T)
    addp((np.asarray(inputs["ll2_b"], f32) * ISQ2).reshape(MT_OUT, 128).T)
    c["prm"] = np.ascontiguousarray(np.concatenate(cols, axis=1))

    in_maps = []
    for cid in range(NCORES):
        xc = x[cid * BC:(cid + 1) * BC]  # [BC, 256, 8]
        xt = np.transpose(xc, (1, 2, 0)).reshape(KT_IN, 128, NB, BC)
        xt = xt.transpose(1, 0, 2, 3)  # [128, kt, 8, BC]
        m = dict(c)
        m["xT"] = np.ascontiguousarray(xt).reshape(128, -1).astype(f16)
        in_maps.append(m)
    return in_maps


def assemble(results):
    out = np.empty((B, NOUT, NB), np.float32)
    for cid in range(NCORES):
        od = np.asarray(results[cid]["outd"]).reshape(128, NB, BC)
        out[cid * BC:(cid + 1) * BC] = od.transpose(2, 0, 1)
    return out


# ----------------------------------------------------------------------------
# Device program (identical on all 8 cores)
# ----------------------------------------------------------------------------
def build_program():
    import concourse.mybir as mybir
    import concourse.tile as tile
    from concourse import bacc

    dt = mybir.dt
    AF = mybir.ActivationFunctionType
    OP = mybir.AluOpType

    nc = bacc.Bacc("TRN2", target_bir_lowering=False, debug=False,
                   num_devices=NCORES)

    def din(name, w, dtype=dt.float16):
        return nc.dram_tensor(name, [128, w], dtype,
                              kind="ExternalInput").ap()

    xT = din("xT", KT_IN * NB * BC)
    lr1w = din("lr1w", KT_IN * 4 * NIN)
    ll1w = din("ll1w", KT_IN * 4 * HID)
    w1w = din("w1w", KT_IN * NPATHS * HID)
    lrgw = din("lrgw", KT_HID * 4 * HID)
    llgw = din("llgw", KT_HID * 4 * HID)
    lr2w = din("lr2w", KT_HID * 4 * HID)
    w2w = din("w2w", KT_HID * NPATHS * NOUT)
    w2n = din("w2n", KT_HID * NNEG * NOUT)
    ll2w = din("ll2w", KT_HID * 4 * NOUT)
    dwg = din("dwg", MT_HID * NPATHS * 128)
    dwgn = din("dwgn", MT_HID * NNEG * 128)
    prm = din("prm", 121, dt.float32)
    outd = nc.dram_tensor("outd", [128, NB * BC], dt.float32,
                          kind="ExternalOutput").ap()

    P = lambda j: slice(j * BC, (j + 1) * BC)
    GSL = [slice(j0 * BC, j1 * BC) for (j0, j1) in GRADE_SLICES]

    with tile.TileContext(nc) as tc:
        top = contextlib.ExitStack()
        with top:
            ppool = top.enter_context(tc.tile_pool(name="params", bufs=1))
            auxpool = top.enter_context(tc.tile_pool(name="aux", bufs=2))
            npool = top.enter_context(tc.tile_pool(name="nsc", bufs=2))
            qpool = top.enter_context(tc.tile_pool(name="q", bufs=2))
            pspool = top.enter_context(
                tc.tile_pool(name="psum", bufs=8, space="PSUM"))
            hpool = top.enter_context(tc.tile_pool(name="hacts", bufs=1))
            w3pool = top.enter_context(tc.tile_pool(name="w_s3", bufs=1))

            # ---- persistent weight loads (stage-3 linear; own SBUF) ----
            def lin_tile(pool, name, src, nkt, mtot):
                t = pool.tile([128, nkt * 4 * mtot], dt.float16,
                              tag=name, name=name)
                nc.sync.dma_start(t[:], src)

                def sl(kt, g, mt):
                    base = (kt * 4 + g) * mtot + mt * 128
                    return t[:, base:base + 128]
                return sl

            # params first (tiny), then x, then stage-1 weights
            prmt = ppool.tile([128, 121], dt.float32, tag="prm", name="prm")
            nc.sync.dma_start(prmt[:], prm)
            dumt = ppool.tile([128, 1], dt.float16, tag="dum", name="dum")

            def preload_act(func, dep=None):
                """Dummy activation so the ACT table-set load happens off
                the critical path (under a matmul burst). `dep` sequences
                the load after the previous set's last consumer."""
                src = prmt[:, 0:1] if dep is None else dep[:, 0:1]
                nc.scalar.activation(dumt[:], src, func)
            PN1, PNG, PN2, PACT, PB1, PBG, PB2 = 0, 16, 48, 80, 112, 116, 120

            def psl(base, u, w=4):
                return prmt[:, base + 8 * u:base + 8 * u + w]

            n1sat = {u: psl(PN1, u) for u in range(KT_IN)}
            n1cbt = {u: prmt[:, PN1 + 8 * u + 4:PN1 + 8 * u + 8]
                     for u in range(KT_IN)}
            ngsat = {u: psl(PNG, u) for u in range(KT_HID)}
            ngcbt = {u: prmt[:, PNG + 8 * u + 4:PNG + 8 * u + 8]
                     for u in range(KT_HID)}
            n2sat = {u: psl(PN2, u) for u in range(KT_HID)}
            n2cbt = {u: prmt[:, PN2 + 8 * u + 4:PN2 + 8 * u + 8]
                     for u in range(KT_HID)}
            actat = {u: psl(PACT, u) for u in range(MT_HID)}
            actbt = {u: prmt[:, PACT + 8 * u + 4:PACT + 8 * u + 8]
                     for u in range(MT_HID)}
            b1t = {u: prmt[:, PB1 + u:PB1 + u + 1] for u in range(MT_HID)}
            bgt = {u: prmt[:, PBG + u:PBG + u + 1] for u in range(MT_HID)}
            b2t = {0: prmt[:, PB2:PB2 + 1]}

            GW = [1, 3, 3, 1]

            class RegionEmitter:
                """start on first / stop on last matmul per psum BANK."""

                def __init__(self, totals):
                    self.totals = dict(totals)
                    self.seen = {}

                def mm(self, reg, dst, lhs, rhs):
                    i = self.seen.get(reg, 0)
                    nc.tensor.matmul(dst, lhs, rhs, start=(i == 0),
                                     stop=(i == self.totals[reg] - 1))
                    self.seen[reg] = i + 1

                def done(self):
                    assert self.seen == self.totals, (self.seen, self.totals)

            def alloc_reg(nm):
                """Two psum banks: A = blades 0..3, B = blades 4..7."""
                a = pspool.tile([128, 4 * BC], dt.float32, tag="bk",
                                name=f"psA_{nm}")
                b = pspool.tile([128, 4 * BC], dt.float32, tag="bk",
                                name=f"psB_{nm}")
                return (a, b)

            # (bank_idx, col offset within bank) for each grade
            GOFF = [(0, 0), (0, BC), (1, 0), (1, 3 * BC)]

            def plane_sel(qpl, o0, L, st):
                if L == 1:
                    return qpl[:, o0:o0 + 1, :]
                last = o0 + st * (L - 1)
                stop = last + 1 if st > 0 else (last - 1 if last >= 1 else None)
                return qpl[:, o0:stop:st, :]

            def build_q(xt, xrt):
                """Mega product tile Q[i*8+k] = x_i * xr_k, [128, 8192]."""
                q = qpool.tile([128, 64 * BC], dt.float16, tag="Q", name="Q")
                for half in range(2):
                    i0 = half * 4
                    a = xt[:, i0 * BC:(i0 + 4) * BC].rearrange(
                        "p (i u b) -> p i u b", i=4, u=1).broadcast_to(
                        [128, 4, 8, BC])
                    bb = xrt[:].rearrange(
                        "p (u k b) -> p u k b", u=1, k=8).broadcast_to(
                        [128, 4, 8, BC])
                    dst = q[:, i0 * 8 * BC:(i0 + 4) * 8 * BC].rearrange(
                        "p (i k b) -> p i k b", i=4, k=8)
                    nc.vector.tensor_mul(dst, a, bb)
                return q

            def lin_mms(em, reg, wsl, xts, kts, mt):
                for kt in kts:
                    for g in range(4):
                        bk, off = GOFF[g]
                        em.mm(bk, reg[bk][:, off:off + GW[g] * BC],
                              wsl(kt, g, mt), xts[kt][:, GSL[g]])

            # GP sets in plane order: with subtile deps, the first matmuls
            # only need the first half of the Q tile
            GP_SETS_ORDERED = sorted(
                ((s[2], g, t, s) for g in range(4)
                 for (t, s) in GP_SETS_BY_GRADE[g]),
                key=lambda x: x[0])

            def gp_mms(em, reg, wsl, wsln, qpl, kt, mt):
                for (_, g, t, (j0, L, o0, st, sgn)) in GP_SETS_ORDERED:
                    bk, boff = GOFF[g]
                    gbase = GRADE_SLICES[g][0]
                    lhs = (wsl if sgn > 0 else wsln)(kt, t, mt)
                    r0 = boff + (j0 - gbase) * BC
                    em.mm(bk, reg[bk][:, r0:r0 + L * BC],
                          lhs, plane_sel(qpl, o0, L, st))

            def evac(reg, raw, bias=None):
                if bias is None:
                    nc.scalar.copy(raw[:, 0:4 * BC], reg[0][:])
                else:
                    nc.scalar.activation(raw[:, 0:BC], reg[0][:, 0:BC],
                                         AF.Identity, bias=bias)
                    nc.scalar.copy(raw[:, BC:4 * BC], reg[0][:, BC:4 * BC])
                nc.scalar.copy(raw[:, 4 * BC:], reg[1][:])

            def normalize(raw, out, sat, cbt, reg):
                sqw = npool.tile([128, 6 * BC], dt.float16, tag="sqw",
                                 name="sqw")
                qw = npool.tile([128, 4 * BC], dt.float16, tag="qw",
                                name="qw")
                # square straight from psum, in parallel with the evac
                nc.scalar.activation(qw[:, 0:BC], reg[0][:, 0:BC], AF.Square)
                nc.scalar.activation(sqw[:, 0:3 * BC], reg[0][:, BC:4 * BC],
                                     AF.Square)
                nc.scalar.activation(sqw[:, 3 * BC:], reg[1][:, 0:3 * BC],
                                     AF.Square)
                nc.scalar.activation(qw[:, 3 * BC:], reg[1][:, 3 * BC:],
                                     AF.Square)
                sqp = sqw[:].rearrange("p (pl b) -> p pl b", pl=6)
                qp = qw[:, BC:3 * BC].rearrange("p (pl b) -> p pl b", pl=2)
                nc.vector.tensor_add(qp, sqp[:, 0:4:3, :], sqp[:, 1:5:3, :])
                nc.vector.tensor_add(qp, qp, sqp[:, 2:6:3, :])
                nrmw = npool.tile([128, 4 * BC], dt.float16, tag="nrmw",
                                  name="nrmw")
                nc.scalar.activation(nrmw[:], qw[:], AF.Sqrt)
                dw = npool.tile([128, 4 * BC], dt.float32, tag="dw",
                                name="dw", bufs=1)
                for g in range(4):
                    nc.vector.tensor_scalar(dw[:, g * BC:(g + 1) * BC],
                                            nrmw[:, g * BC:(g + 1) * BC],
                                            sat[:, g:g + 1], cbt[:, g:g + 1],
                                            OP.mult, OP.add)
                rw = npool.tile([128, 4 * BC], dt.float32, tag="rw",
                                name="rw", bufs=1)
                nc.vector.reciprocal_approx_fast(rw[:], dw[:])
                r16 = npool.tile([128, 4 * BC], dt.float16, tag="r16",
                                 name="r16")
                nc.vector.tensor_copy(r16[:], rw[:])
                nc.vector.tensor_mul(out[:, P(0)], raw[:, P(0)],
                                     r16[:, 0:BC])
                bb = r16[:, BC:3 * BC].rearrange(
                    "p (g u b) -> p g u b", g=2, u=1).broadcast_to(
                    [128, 2, 3, BC])
                nc.vector.tensor_mul(
                    out[:, BC:7 * BC].rearrange(
                        "p (g i b) -> p g i b", g=2, i=3),
                    raw[:, BC:7 * BC].rearrange(
                        "p (g i b) -> p g i b", g=2, i=3), bb)
                nc.vector.tensor_mul(out[:, P(7)], raw[:, P(7)],
                                     r16[:, 3 * BC:])

            def mv_silu(raw, out, at, bt, reg):
                sqw = npool.tile([128, 6 * BC], dt.float16, tag="sqw",
                                 name="sqw")
                sq7 = npool.tile([128, BC], dt.float16, tag="sq7",
                                 name="sq7")
                # blades 1..7 are bias-free so squaring from psum is exact
                nc.scalar.activation(sqw[:, 0:3 * BC], reg[0][:, BC:4 * BC],
                                     AF.Square)
                nc.scalar.activation(sqw[:, 3 * BC:], reg[1][:, 0:3 * BC],
                                     AF.Square)
                nc.scalar.activation(sq7[:], reg[1][:, 3 * BC:], AF.Square)
                q12 = npool.tile([128, 2 * BC], dt.float16, tag="q12",
                                 name="q12")
                sqp = sqw[:].rearrange("p (pl b) -> p pl b", pl=6)
                qp = q12[:].rearrange("p (pl b) -> p pl b", pl=2)
                nc.vector.tensor_add(qp, sqp[:, 0:4:3, :], sqp[:, 1:5:3, :])
                nc.vector.tensor_add(qp, qp, sqp[:, 2:6:3, :])
                invs = [raw[:, P(0)], q12[:, 0:BC], q12[:, BC:], sq7[:]]
                gw = npool.tile([128, 4 * BC], dt.float16, tag="gw",
                                name="gw")
                for g in range(4):
                    nc.scalar.activation(gw[:, g * BC:(g + 1) * BC],
                                         invs[g], AF.Sigmoid,
                                         bias=bt[:, g:g + 1],
                                         scale=at[:, g:g + 1])
                nc.vector.tensor_mul(out[:, P(0)], raw[:, P(0)],
                                     gw[:, 0:BC])
                bb = gw[:, BC:3 * BC].rearrange(
                    "p (g u b) -> p g u b", g=2, u=1).broadcast_to(
                    [128, 2, 3, BC])
                nc.vector.tensor_mul(
                    out[:, BC:7 * BC].rearrange(
                        "p (g i b) -> p g i b", g=2, i=3),
                    raw[:, BC:7 * BC].rearrange(
                        "p (g i b) -> p g i b", g=2, i=3), bb)
                nc.vector.tensor_mul(out[:, P(7)], raw[:, P(7)],
                                     gw[:, 3 * BC:])

            Ht, H2t, HR2t = {}, {}, {}

            # ================= stages 1-2 ===================================
            with tc.tile_pool(name="xacts", bufs=1) as xpool, \
                 tc.tile_pool(name="w_s1", bufs=1) as w1pool:
                Xall = xpool.tile([128, KT_IN * NB * BC], dt.float16,
                                  tag="X", name="X")
                nc.sync.dma_start(Xall[:], xT)
                Xt = {kt: Xall[:, kt * NB * BC:(kt + 1) * NB * BC]
                      for kt in range(KT_IN)}
                lr1t = lin_tile(w1pool, "lr1", lr1w, KT_IN, NIN)
                ll1t = lin_tile(w1pool, "ll1", ll1w, KT_IN, HID)
                w1t_ = w1pool.tile([128, KT_IN * NPATHS * HID], dt.float16,
                                   tag="w1", name="w1")
                nc.sync.dma_start(w1t_[:], w1w)
                # negated copies built on-chip (ScalarE) to keep the
                # critical early DMA path lean
                w1nt_ = w1pool.tile([128, KT_IN * NNEG * HID], dt.float16,
                                    tag="w1nn", name="w1nn")
                for kt in range(KT_IN):
                    for (t0, ln) in NEG_RUNS:
                        sb = (kt * NPATHS + t0) * HID
                        db = (kt * NNEG + NEG_SLOT[t0]) * HID
                        nc.scalar.activation(
                            w1nt_[:, db:db + ln * HID],
                            w1t_[:, sb:sb + ln * HID],
                            AF.Identity, scale=-1.0)
                # stage-3 linear weights: own SBUF (w3pool entered at top),
                # DMA emitted after the stage-1 critical loads
                lrgt = lin_tile(w3pool, "lrg", lrgw, KT_HID, HID)
                llgt = lin_tile(w3pool, "llg", llgw, KT_HID, HID)

                def w1sl(kt, t, mt):
                    base = (kt * NPATHS + t) * HID + mt * 128
                    return w1t_[:, base:base + 128]

                def w1sln(kt, t, mt):
                    base = (kt * NNEG + NEG_SLOT[t]) * HID + mt * 128
                    return w1nt_[:, base:base + 128]

                preload_act(AF.Sqrt)  # load sqrt table during initial DMA

                # --- S1: xr = normalization(lr1(x)) ---
                XRt, regs1 = {}, {}
                for mt in range(MT_IN):
                    regs1[mt] = alloc_reg(f"lr1_{mt}")
                    em = RegionEmitter({0: KT_IN * 2, 1: KT_IN * 2})
                    lin_mms(em, regs1[mt], lr1t, Xt, range(KT_IN), mt)
                    em.done()
                    raw = auxpool.tile([128, NB * BC], dt.float16,
                                       tag="raw", name="raw")
                    evac(regs1[mt], raw)
                    xr = xpool.tile([128, NB * BC], dt.float16,
                                    tag=f"XR_{mt}", name=f"XR_{mt}")
                    normalize(raw, xr, n1sat[mt], n1cbt[mt], regs1[mt])
                    XRt[mt] = xr
                preload_act(AF.Sigmoid, XRt[1])  # for S2 silus

                Qs = {kt: build_q(Xt[kt], XRt[kt]) for kt in range(KT_IN)}
                Qpl = {kt: Qs[kt][:].rearrange("p (pl b) -> p pl b", pl=64)
                       for kt in range(KT_IN)}

                # --- S2: h = silu((ll1(x) + fcgp(x, xr, w1)) / sqrt2) ---
                regs2, ems2 = {}, {}
                tot2 = {b: KT_IN * (2 + NSETS_BANK[b]) for b in (0, 1)}
                for mt in range(3):     # early regions: lin first (X-dep)
                    regs2[mt] = alloc_reg(f"h_{mt}")
                    ems2[mt] = RegionEmitter(tot2)
                    lin_mms(ems2[mt], regs2[mt], ll1t, Xt, range(KT_IN), mt)
                for kt in range(KT_IN):
                    for mt in range(3):
                        gp_mms(ems2[mt], regs2[mt], w1sl, w1sln, Qpl[kt],
                               kt, mt)
                for mt in range(3):
                    ems2[mt].done()
                    raw = auxpool.tile([128, NB * BC], dt.float16,
                                       tag="raw", name="raw")
                    evac(regs2[mt], raw, bias=b1t[mt])
                    h = hpool.tile([128, NB * BC], dt.float16,
                                   tag=f"H_{mt}", name=f"H_{mt}")
                    mv_silu(raw, h, actat[mt], actbt[mt], regs2[mt])
                    Ht[mt] = h
                # mt=3 last (its region slot frees after lr1_0 retires)
                regs2[3] = alloc_reg("h_3")
                em = RegionEmitter(tot2)
                lin_mms(em, regs2[3], ll1t, Xt, range(KT_IN), 3)
                for kt in range(KT_IN):
                    gp_mms(em, regs2[3], w1sl, w1sln, Qpl[kt], kt, 3)
                em.done()
                raw = auxpool.tile([128, NB * BC], dt.float16,
                                   tag="raw", name="raw")
                evac(regs2[3], raw, bias=b1t[3])
                h = hpool.tile([128, NB * BC], dt.float16,
                               tag="H_3", name="H_3")
                mv_silu(raw, h, actat[3], actbt[3], regs2[3])
                Ht[3] = h
                preload_act(AF.Sqrt, Ht[3])  # for S3 normalizes

            # ================= stages 3-6 ===================================
            with tc.tile_pool(name="hracts", bufs=1) as hrpool, \
                 tc.tile_pool(name="w_s5", bufs=1) as w5pool:
                # late weights into the space stage-1 weights vacated
                dt_ = w5pool.tile([128, MT_HID * NPATHS * 128], dt.float16,
                                  tag="dwg", name="dwg")
                nc.sync.dma_start(dt_[:], dwg)
                dn_ = w5pool.tile([128, MT_HID * NNEG * 128], dt.float16,
                                  tag="dwgn", name="dwgn")
                nc.sync.dma_start(dn_[:], dwgn)
                lr2t = lin_tile(w5pool, "lr2", lr2w, KT_HID, HID)
                w2t_ = w5pool.tile([128, KT_HID * NPATHS * NOUT], dt.float16,
                                   tag="w2", name="w2")
                nc.sync.dma_start(w2t_[:], w2w)
                w2nt_ = w5pool.tile([128, KT_HID * NNEG * NOUT], dt.float16,
                                    tag="w2nn", name="w2nn")
                nc.sync.dma_start(w2nt_[:], w2n)
                ll2t = lin_tile(w5pool, "ll2", ll2w, KT_HID, NOUT)

                def dwgsl(ct, t, mt=None):
                    base = (ct * NPATHS + t) * 128
                    return dt_[:, base:base + 128]

                def dwgsln(ct, t, mt=None):
                    base = (ct * NNEG + NEG_SLOT[t]) * 128
                    return dn_[:, base:base + 128]

                def w2sl(kt, t, mt):
                    base = (kt * NPATHS + t) * NOUT + mt * 128
                    return w2t_[:, base:base + 128]

                def w2sln(kt, t, mt):
                    base = (kt * NNEG + NEG_SLOT[t]) * NOUT + mt * 128
                    return w2nt_[:, base:base + 128]

                # --- S3: hr = normalization(lrg(h)) ---
                # early kts for mt 0..2 overlap H_3's silu tail
                HRt, regs3, ems3 = {}, {}, {}
                tot3 = {0: KT_HID * 2, 1: KT_HID * 2}
                for mt in range(3):
                    regs3[mt] = alloc_reg(f"lrg_{mt}")
                    ems3[mt] = RegionEmitter(tot3)
                    lin_mms(ems3[mt], regs3[mt], lrgt, Ht, range(3), mt)
                for mt in range(3):
                    lin_mms(ems3[mt], regs3[mt], lrgt, Ht, [3], mt)
                    ems3[mt].done()
                    raw = auxpool.tile([128, NB * BC], dt.float16,
                                       tag="raw", name="raw")
                    evac(regs3[mt], raw)
                    hr = hrpool.tile([128, NB * BC], dt.float16,
                                     tag=f"HR_{mt}", name=f"HR_{mt}")
                    normalize(raw, hr, ngsat[mt], ngcbt[mt], regs3[mt])
                    HRt[mt] = hr
                regs3[3] = alloc_reg("lrg_3")
                em3 = RegionEmitter(tot3)
                lin_mms(em3, regs3[3], lrgt, Ht, range(KT_HID), 3)
                em3.done()
                raw = auxpool.tile([128, NB * BC], dt.float16,
                                   tag="raw", name="raw")
                evac(regs3[3], raw)
                hr = hrpool.tile([128, NB * BC], dt.float16,
                                 tag="HR_3", name="HR_3")
                normalize(raw, hr, ngsat[3], ngcbt[3], regs3[3])
                HRt[3] = hr
                preload_act(AF.Sigmoid, HRt[3])  # for S4 silus

                # --- S4: h2 = silu((llg(h) + cw_gp(h, hr, wg)) / sqrt2) ---
                regs4, ems4 = {}, {}
                tot4 = {b: KT_HID * 2 + NSETS_BANK[b] for b in (0, 1)}
                for mt in range(3):
                    regs4[mt] = alloc_reg(f"h2_{mt}")
                    ems4[mt] = RegionEmitter(tot4)
                    lin_mms(ems4[mt], regs4[mt], llgt, Ht, range(3), mt)
                for mt in range(4):
                    if mt < 3:
                        lin_mms(ems4[mt], regs4[mt], llgt, Ht, [3], mt)
                    else:
                        regs4[3] = alloc_reg("h2_3")
                        ems4[3] = RegionEmitter(tot4)
                        lin_mms(ems4[3], regs4[3], llgt, Ht, range(KT_HID), 3)
                    q = build_q(Ht[mt], HRt[mt])
                    qpl = q[:].rearrange("p (pl b) -> p pl b", pl=64)
                    gp_mms(ems4[mt], regs4[mt], dwgsl, dwgsln, qpl, mt, None)
                    ems4[mt].done()
                    raw = auxpool.tile([128, NB * BC], dt.float16,
                                       tag="raw", name="raw")
                    evac(regs4[mt], raw, bias=bgt[mt])
                    h2 = hpool.tile([128, NB * BC], dt.float16,
                                    tag=f"H2_{mt}", name=f"H2_{mt}")
                    mv_silu(raw, h2, actat[mt], actbt[mt], regs4[mt])
                    H2t[mt] = h2
                preload_act(AF.Sqrt, H2t[3])  # for S5 normalizes

                # --- S5: hr2 = normalization(lr2(h2)) ---
                regs5, ems5 = {}, {}
                for mt in range(3):
                    regs5[mt] = alloc_reg(f"lr2_{mt}")
                    ems5[mt] = RegionEmitter(tot3)
                    lin_mms(ems5[mt], regs5[mt], lr2t, H2t, range(3), mt)
                for mt in range(3):
                    lin_mms(ems5[mt], regs5[mt], lr2t, H2t, [3], mt)
                    ems5[mt].done()
                    raw = auxpool.tile([128, NB * BC], dt.float16,
                                       tag="raw", name="raw")
                    evac(regs5[mt], raw)
                    hr2 = hpool.tile([128, NB * BC], dt.float16,
                                     tag=f"HR2_{mt}", name=f"HR2_{mt}")
                    normalize(raw, hr2, n2sat[mt], n2cbt[mt], regs5[mt])
                    HR2t[mt] = hr2
                regs5[3] = alloc_reg("lr2_3")
                em5 = RegionEmitter(tot3)
                lin_mms(em5, regs5[3], lr2t, H2t, range(KT_HID), 3)
                em5.done()
                raw = auxpool.tile([128, NB * BC], dt.float16,
                                   tag="raw", name="raw")
                evac(regs5[3], raw)
                hr2 = hpool.tile([128, NB * BC], dt.float16,
                                 tag="HR2_3", name="HR2_3")
                normalize(raw, hr2, n2sat[3], n2cbt[3], regs5[3])
                HR2t[3] = hr2

                # --- S6: out = (ll2(h2) + fcgp(h2, hr2, w2)) / sqrt2 ---
                reg6 = alloc_reg("out")
                tot6 = {b: KT_HID * (2 + NSETS_BANK[b]) for b in (0, 1)}
                em6 = RegionEmitter(tot6)
                lin_mms(em6, reg6, ll2t, H2t, range(KT_HID), 0)
                for kt in range(KT_HID):
                    q = build_q(H2t[kt], HR2t[kt])
                    qpl = q[:].rearrange("p (pl b) -> p pl b", pl=64)
                    gp_mms(em6, reg6, w2sl, w2sln, qpl, kt, 0)
                em6.done()
                outs = auxpool.tile([128, NB * BC], dt.float32, tag="outs",
                                    name="outs", bufs=1)
                evac(reg6, outs, bias=b2t[0])
                nc.sync.dma_start(outd, outs[:])

    nc.compile()
    return nc


_PROGRAM = None


def _get_program():
    global _PROGRAM
    if _PROGRAM is None:
        _PROGRAM = build_program()
    return _PROGRAM


def kernel(**inputs):
    from concourse.bass_utils import run_bass_kernel_spmd

    nc = _get_program()
    in_maps = prep_in_maps(inputs)
    res = run_bass_kernel_spmd(nc, in_maps, core_ids=list(range(NCORES)))
    return assemble(res.results)


if __name__ == "__main__":
    nmm = sum(len(TERM_SETS[t]) for t in range(NPATHS))
    print("NEG_TRIPLES:", NEG_TRIPLES)
    print("term-set MMs per (kt,mt):", nmm)
    print("NSETS_BANK:", NSETS_BANK)


# revision 23
# speedup vs baseline: 1.0237x; 1.0237x over previous
"""Trainium2 Bass kernel for nn_CliffordFourierHead (CGENN-style Clifford net).

Network (per reference): B=1024, IN=256, HID=512, OUT=128, Cl(3,0), 8 blades.
  fcgp1 -> MVSiLU -> channel-wise steerable GP -> MVSiLU -> fcgp2

Strategy (v3):
  - Pure batch data-parallelism over 8 NeuronCores (128 batch rows each).
  - Channels on partitions, batch on free dim; an activation is 8 blade
    planes packed into one [128, 8*128] SBUF tile per channel-tile.
  - Geometric products: ONE wide DVE op pair builds a mega product tile
    Q[i,k] = x_i * xr_k; the Cayley contraction is absorbed into TensorE
    matmuls that accumulate into PSUM (negative signs via negated weight
    copies).
  - PSUM regions packed 2 banks per output tile: bank A = blades 0..3,
    bank B = blades 4..7 -> 4 regions in flight -> cross-phase overlap.
  - Emission order interleaves each phase's early-kt matmuls under the
    previous phase's normalize/silu tail so the PE never starves.
  - All weights pre-laid-out host-side as [128, W] contiguous slabs ->
    single fat DMA descriptors; stage-3 linear weights get their own SBUF
    region up front so their DMA overlaps stage-1 compute.
  - fp16 on-chip compute, fp32 PSUM/params.

Self-contained: shapes and the Cl(3,0) Cayley table are derived inline.
"""

import contextlib
import math

import numpy as np

NCORES = 8
B, NIN, HID, NOUT = 1024, 256, 512, 128
BC = B // NCORES  # 128 batch rows per core
NB = 8
KT_IN, KT_HID = NIN // 128, HID // 128  # 2, 4
MT_IN, MT_HID, MT_OUT = NIN // 128, HID // 128, NOUT // 128  # 2, 4, 1
GRADE_SLICES = [(0, 1), (1, 4), (4, 7), (7, 8)]
GRADE_OF = [0, 1, 1, 1, 2, 2, 2, 3]
EPS = 1e-6
ISQ2 = 1.0 / math.sqrt(2.0)


def _build_cayley():
    masks = sorted(range(NB), key=lambda m: (bin(m).count("1"), m))
    pos = {m: i for i, m in enumerate(masks)}
    cay = np.zeros((NB, NB, NB), dtype=np.float32)
    for i, mi in enumerate(masks):
        for k, mk in enumerate(masks):
            a, s = mi >> 1, 0
            while a:
                s += bin(a & mk).count("1")
                a >>= 1
            cay[i, pos[mi ^ mk], k] = -1.0 if (s & 1) else 1.0
    triples = []
    for gi in range(4):
        for gj in range(4):
            for gk in range(4):
                (i0, i1), (j0, j1), (k0, k1) = (
                    GRADE_SLICES[gi], GRADE_SLICES[gj], GRADE_SLICES[gk])
                if np.any(cay[i0:i1, j0:j1, k0:k1] != 0):
                    triples.append((gi, gj, gk))
    return cay, triples


CAY, TRIPLES = _build_cayley()
NPATHS = len(TRIPLES)  # 20

# Per triple t: {j: [(i, k, sign), ...]}
TRIPLE_TERMS = []
for t, (gi, gj, gk) in enumerate(TRIPLES):
    (i0, i1), (k0, k1) = GRADE_SLICES[gi], GRADE_SLICES[gk]
    d = {}
    for i in range(i0, i1):
        for k in range(k0, k1):
            j = int(np.nonzero(CAY[i, :, k])[0][0])
            if GRADE_SLICES[gj][0] <= j < GRADE_SLICES[gj][1]:
                d.setdefault(j, []).append((i, k, float(CAY[i, j, k])))
    TRIPLE_TERMS.append(d)


def _build_term_sets():
    """Per triple: list of matmul term-sets (j0, L, plane0, plane_step, sign).

    A term-set is a run of consecutive output blades j0..j0+L-1, one product
    plane each, uniform Cayley sign, arithmetic plane offsets (plane = i*8+k
    in the mega product tile) -> one matmul with a strided rhs plane AP.
    """
    all_sets = []
    for t in range(NPATHS):
        terms = []
        for j, lst in TRIPLE_TERMS[t].items():
            for (i, k, s) in lst:
                terms.append((j, i * 8 + k, s))
        sets = []
        for sgn in (1.0, -1.0):
            pool = sorted(x for x in terms if x[2] == sgn)
            while pool:
                j0, o0, _ = pool.pop(0)
                run = [(j0, o0)]
                step = None
                while True:
                    pick = None
                    for c in pool:
                        if c[0] != run[-1][0] + 1:
                            continue
                        st = c[1] - run[-1][1]
                        if step is None or st == step:
                            pick, pstep = c, st
                            break
                    if pick is None:
                        break
                    step = pstep
                    pool.remove(pick)
                    run.append((pick[0], pick[1]))
                sets.append((run[0][0], len(run), run[0][1], step or 0, sgn))
        all_sets.append(sets)
    return all_sets


TERM_SETS = _build_term_sets()
NEG_TRIPLES = sorted({t for t in range(NPATHS)
                      if any(s[4] < 0 for s in TERM_SETS[t])})
NEG_SLOT = {t: n for n, t in enumerate(NEG_TRIPLES)}
NNEG = len(NEG_TRIPLES)
NEG_RUNS = []
_i = 0
while _i < NNEG:
    _j = _i
    while _j + 1 < NNEG and NEG_TRIPLES[_j + 1] == NEG_TRIPLES[_j] + 1:
        _j += 1
    NEG_RUNS.append((NEG_TRIPLES[_i], _j - _i + 1))
    _i = _j + 1

# GP term-sets grouped by output grade; bank 0 = grades {0,1}, 1 = {2,3}
GP_SETS_BY_GRADE = {g: [(t, s) for t in range(NPATHS)
                        if TRIPLES[t][1] == g
                        for s in TERM_SETS[t]]
                    for g in range(4)}
NSETS_BANK = {0: len(GP_SETS_BY_GRADE[0]) + len(GP_SETS_BY_GRADE[1]),
              1: len(GP_SETS_BY_GRADE[2]) + len(GP_SETS_BY_GRADE[3])}


# ----------------------------------------------------------------------------
# Host-side prep (all tensors laid out [128, W] contiguous)
# ----------------------------------------------------------------------------
def prep_in_maps(inputs):
    f16, f32 = np.float16, np.float32

    def lin_w(w, scale=1.0):
        # [m, n, 4] -> [128, kt*4*m]
        m, n, _ = np.asarray(w).shape
        wt = np.transpose(np.asarray(w, f32), (1, 2, 0))  # [n, 4, m]
        wt = wt.reshape(n // 128, 128, 4, m).transpose(1, 0, 2, 3)
        return np.ascontiguousarray(wt * scale).reshape(128, -1).astype(f16)

    def gp_w(w, scale):
        # [m, n, 20] -> pos [128, kt*20*m], neg [128, kt*12*m]
        m, n, _ = np.asarray(w).shape
        wt = np.transpose(np.asarray(w, f32), (2, 1, 0)) * scale  # [20, n, m]
        wt = wt.reshape(NPATHS, n // 128, 128, m).transpose(2, 1, 0, 3)
        pos = np.ascontiguousarray(wt).reshape(128, -1).astype(f16)
        neg = np.ascontiguousarray(-wt[:, :, NEG_TRIPLES, :]).reshape(
            128, -1).astype(f16)
        return pos, neg

    def sig(a):
        return 1.0 / (1.0 + np.exp(-np.asarray(a, f32)))

    x = np.asarray(inputs["x"], f32)

    c = {}
    c["lr1w"] = lin_w(inputs["lr1_w"])
    c["ll1w"] = lin_w(inputs["ll1_w"], ISQ2)
    c["lrgw"] = lin_w(inputs["lrg_w"])
    c["llgw"] = lin_w(inputs["llg_w"], ISQ2)
    c["lr2w"] = lin_w(inputs["lr2_w"])
    c["ll2w"] = lin_w(inputs["ll2_w"], ISQ2)
    c["w1w"], _ = gp_w(inputs["w1"], ISQ2)
    c["w2w"], c["w2n"] = gp_w(inputs["w2"], ISQ2)

    # channel-wise GP weights as diagonal matrices [128, ct*20*128] (+neg)
    wg = np.asarray(inputs["wg"], f32) * ISQ2  # [HID, 20]
    dwg = np.zeros((MT_HID, NPATHS, 128, 128), f32)
    idx = np.arange(128)
    for t in range(NPATHS):
        wv = wg[:, t].reshape(MT_HID, 128)
        for ct in range(MT_HID):
            dwg[ct, t, idx, idx] = wv[ct]
    dwgp = dwg.transpose(2, 0, 1, 3)  # [128, ct, 20, 128]
    c["dwg"] = np.ascontiguousarray(dwgp).reshape(128, -1).astype(f16)
    c["dwgn"] = np.ascontiguousarray(
        -dwgp[:, :, NEG_TRIPLES, :]).reshape(128, -1).astype(f16)

    cols = []   # list of [128, w] blocks; order must match device PARAM map

    def addp(arr):
        cols.append(np.asarray(arr, f32).reshape(128, -1))

    for nm, a, kt in (("n1", inputs["n1_a"], KT_IN),
                      ("ng", inputs["ng_a"], KT_HID),
                      ("n2", inputs["n2_a"], KT_HID)):
        sa = sig(a).reshape(kt, 128, 4)
        cb = (1.0 + EPS) - sa
        for u in range(kt):
            addp(sa[u])
            addp(cb[u])
    aa = np.asarray(inputs["act_a"], f32).reshape(MT_HID, 128, 4)
    ab = np.asarray(inputs["act_b"], f32).reshape(MT_HID, 128, 4)
    for u in range(MT_HID):
        addp(aa[u])
        addp(ab[u])
    addp((np.asarray(inputs["ll1_b"], f32) * ISQ2).reshape(MT_HID, 128).T)
    addp((np.asarray(inputs["llg_b"], f32) * ISQ2).reshape(MT_HID, 128).T)
    addp((np.asarray(inputs["ll2_b"], f32) * ISQ2).reshape(MT_OUT, 128).T)
    c["prm"] = np.ascontiguousarray(np.concatenate(cols, axis=1))

    in_maps = []
    for cid in range(NCORES):
        xc = x[cid * BC:(cid + 1) * BC]  # [BC, 256, 8]
        xt = np.transpose(xc, (1, 2, 0)).reshape(KT_IN, 128, NB, BC)
        xt = xt.transpose(1, 0, 2, 3)  # [128, kt, 8, BC]
        m = dict(c)
        m["xT"] = np.ascontiguousarray(xt).reshape(128, -1).astype(f16)
        in_maps.append(m)
    return in_maps


def assemble(results):
    out = np.empty((B, NOUT, NB), np.float32)
    for cid in range(NCORES):
        od = np.asarray(results[cid]["outd"]).reshape(128, NB, BC)
        out[cid * BC:(cid + 1) * BC] = od.transpose(2, 0, 1)
    return out


# ----------------------------------------------------------------------------
# Device program (identical on all 8 cores)
# ----------------------------------------------------------------------------
def build_program():
    import concourse.mybir as mybir
    import concourse.tile as tile
    from concourse import bacc

    dt = mybir.dt
    AF = mybir.ActivationFunctionType
    OP = mybir.AluOpType

    nc = bacc.Bacc("TRN2", target_bir_lowering=False, debug=False,
                   num_devices=NCORES)

    def din(name, w, dtype=dt.float16):
        return nc.dram_tensor(name, [128, w], dtype,
                              kind="ExternalInput").ap()

    xT = din("xT", KT_IN * NB * BC)
    lr1w = din("lr1w", KT_IN * 4 * NIN)
    ll1w = din("ll1w", KT_IN * 4 * HID)
    w1w = din("w1w", KT_IN * NPATHS * HID)
    lrgw = din("lrgw", KT_HID * 4 * HID)
    llgw = din("llgw", KT_HID * 4 * HID)
    lr2w = din("lr2w", KT_HID * 4 * HID)
    w2w = din("w2w", KT_HID * NPATHS * NOUT)
    w2n = din("w2n", KT_HID * NNEG * NOUT)
    ll2w = din("ll2w", KT_HID * 4 * NOUT)
    dwg = din("dwg", MT_HID * NPATHS * 128)
    dwgn = din("dwgn", MT_HID * NNEG * 128)
    prm = din("prm", 121, dt.float32)
    outd = nc.dram_tensor("outd", [128, NB * BC], dt.float32,
                          kind="ExternalOutput").ap()

    P = lambda j: slice(j * BC, (j + 1) * BC)
    GSL = [slice(j0 * BC, j1 * BC) for (j0, j1) in GRADE_SLICES]

    with tile.TileContext(nc) as tc:
        top = contextlib.ExitStack()
        with top:
            ppool = top.enter_context(tc.tile_pool(name="params", bufs=1))
            auxpool = top.enter_context(tc.tile_pool(name="aux", bufs=2))
            npool = top.enter_context(tc.tile_pool(name="nsc", bufs=2))
            qpool = top.enter_context(tc.tile_pool(name="q", bufs=2))
            pspool = top.enter_context(
                tc.tile_pool(name="psum", bufs=8, space="PSUM"))
            hpool = top.enter_context(tc.tile_pool(name="hacts", bufs=1))
            w3pool = top.enter_context(tc.tile_pool(name="w_s3", bufs=1))

            # ---- persistent weight loads (stage-3 linear; own SBUF) ----
            def lin_tile(pool, name, src, nkt, mtot):
                t = pool.tile([128, nkt * 4 * mtot], dt.float16,
                              tag=name, name=name)
                nc.sync.dma_start(t[:], src)

                def sl(kt, g, mt):
                    base = (kt * 4 + g) * mtot + mt * 128
                    return t[:, base:base + 128]
                return sl

            # params first (tiny), then x, then stage-1 weights
            prmt = ppool.tile([128, 121], dt.float32, tag="prm", name="prm")
            nc.sync.dma_start(prmt[:], prm)
            dumt = ppool.tile([128, 1], dt.float16, tag="dum", name="dum")

            def preload_act(func, dep=None):
                """Dummy activation so the ACT table-set load happens off
                the critical path (under a matmul burst). `dep` sequences
                the load after the previous set's last consumer. Square is
                in every table set (keeps the input nonneg for Sqrt)."""
                if dep is not None:
                    # blade-7 column: written by the phase's final DVE mul,
                    # which depends on the last ScalarE gate of the old set
                    nc.scalar.activation(dumt[:], dep[:, 7 * BC:7 * BC + 1],
                                         AF.Square)
                    nc.scalar.activation(dumt[:], dumt[:], func)
                else:
                    nc.scalar.activation(dumt[:], prmt[:, 0:1], func)
            PN1, PNG, PN2, PACT, PB1, PBG, PB2 = 0, 16, 48, 80, 112, 116, 120

            def psl(base, u, w=4):
                return prmt[:, base + 8 * u:base + 8 * u + w]

            n1sat = {u: psl(PN1, u) for u in range(KT_IN)}
            n1cbt = {u: prmt[:, PN1 + 8 * u + 4:PN1 + 8 * u + 8]
                     for u in range(KT_IN)}
            ngsat = {u: psl(PNG, u) for u in range(KT_HID)}
            ngcbt = {u: prmt[:, PNG + 8 * u + 4:PNG + 8 * u + 8]
                     for u in range(KT_HID)}
            n2sat = {u: psl(PN2, u) for u in range(KT_HID)}
            n2cbt = {u: prmt[:, PN2 + 8 * u + 4:PN2 + 8 * u + 8]
                     for u in range(KT_HID)}
            actat = {u: psl(PACT, u) for u in range(MT_HID)}
            actbt = {u: prmt[:, PACT + 8 * u + 4:PACT + 8 * u + 8]
                     for u in range(MT_HID)}
            b1t = {u: prmt[:, PB1 + u:PB1 + u + 1] for u in range(MT_HID)}
            bgt = {u: prmt[:, PBG + u:PBG + u + 1] for u in range(MT_HID)}
            b2t = {0: prmt[:, PB2:PB2 + 1]}

            GW = [1, 3, 3, 1]

            class RegionEmitter:
                """start on first / stop on last matmul per psum BANK."""

                def __init__(self, totals):
                    self.totals = dict(totals)
                    self.seen = {}

                def mm(self, reg, dst, lhs, rhs):
                    i = self.seen.get(reg, 0)
                    nc.tensor.matmul(dst, lhs, rhs, start=(i == 0),
                                     stop=(i == self.totals[reg] - 1))
                    self.seen[reg] = i + 1

                def done(self):
                    assert self.seen == self.totals, (self.seen, self.totals)

            def alloc_reg(nm):
                """Two psum banks: A = blades 0..3, B = blades 4..7."""
                a = pspool.tile([128, 4 * BC], dt.float32, tag="bk",
                                name=f"psA_{nm}")
                b = pspool.tile([128, 4 * BC], dt.float32, tag="bk",
                                name=f"psB_{nm}")
                return (a, b)

            # (bank_idx, col offset within bank) for each grade
            GOFF = [(0, 0), (0, BC), (1, 0), (1, 3 * BC)]

            def plane_sel(qpl, o0, L, st):
                if L == 1:
                    return qpl[:, o0:o0 + 1, :]
                last = o0 + st * (L - 1)
                stop = last + 1 if st > 0 else (last - 1 if last >= 1 else None)
                return qpl[:, o0:stop:st, :]

            def build_q(xt, xrt):
                """Mega product tile Q[i*8+k] = x_i * xr_k, [128, 8192]."""
                q = qpool.tile([128, 64 * BC], dt.float16, tag="Q", name="Q")
                for half in range(2):
                    i0 = half * 4
                    a = xt[:, i0 * BC:(i0 + 4) * BC].rearrange(
                        "p (i u b) -> p i u b", i=4, u=1).broadcast_to(
                        [128, 4, 8, BC])
                    bb = xrt[:].rearrange(
                        "p (u k b) -> p u k b", u=1, k=8).broadcast_to(
                        [128, 4, 8, BC])
                    dst = q[:, i0 * 8 * BC:(i0 + 4) * 8 * BC].rearrange(
                        "p (i k b) -> p i k b", i=4, k=8)
                    nc.vector.tensor_mul(dst, a, bb)
                return q

            def lin_mms(em, reg, wsl, xts, kts, mt):
                for kt in kts:
                    for g in range(4):
                        bk, off = GOFF[g]
                        em.mm(bk, reg[bk][:, off:off + GW[g] * BC],
                              wsl(kt, g, mt), xts[kt][:, GSL[g]])

            # GP sets in plane order: with subtile deps, the first matmuls
            # only need the first half of the Q tile
            GP_SETS_ORDERED = sorted(
                ((s[2], g, t, s) for g in range(4)
                 for (t, s) in GP_SETS_BY_GRADE[g]),
                key=lambda x: x[0])

            def gp_mms(em, reg, wsl, wsln, qpl, kt, mt):
                for (_, g, t, (j0, L, o0, st, sgn)) in GP_SETS_ORDERED:
                    bk, boff = GOFF[g]
                    gbase = GRADE_SLICES[g][0]
                    lhs = (wsl if sgn > 0 else wsln)(kt, t, mt)
                    r0 = boff + (j0 - gbase) * BC
                    em.mm(bk, reg[bk][:, r0:r0 + L * BC],
                          lhs, plane_sel(qpl, o0, L, st))

            def evac(reg, raw, bias=None):
                if bias is None:
                    nc.scalar.copy(raw[:, 0:4 * BC], reg[0][:])
                else:
                    nc.scalar.activation(raw[:, 0:BC], reg[0][:, 0:BC],
                                         AF.Identity, bias=bias)
                    nc.scalar.copy(raw[:, BC:4 * BC], reg[0][:, BC:4 * BC])
                nc.scalar.copy(raw[:, 4 * BC:], reg[1][:])

            def normalize(raw, out, sat, cbt, reg):
                sqw = npool.tile([128, 6 * BC], dt.float16, tag="sqw",
                                 name="sqw")
                qw = npool.tile([128, 4 * BC], dt.float16, tag="qw",
                                name="qw")
                # square straight from psum, in parallel with the evac
                nc.scalar.activation(qw[:, 0:BC], reg[0][:, 0:BC], AF.Square)
                nc.scalar.activation(sqw[:, 0:3 * BC], reg[0][:, BC:4 * BC],
                                     AF.Square)
                nc.scalar.activation(sqw[:, 3 * BC:], reg[1][:, 0:3 * BC],
                                     AF.Square)
                nc.scalar.activation(qw[:, 3 * BC:], reg[1][:, 3 * BC:],
                                     AF.Square)
                sqp = sqw[:].rearrange("p (pl b) -> p pl b", pl=6)
                qp = qw[:, BC:3 * BC].rearrange("p (pl b) -> p pl b", pl=2)
                nc.vector.tensor_add(qp, sqp[:, 0:4:3, :], sqp[:, 1:5:3, :])
                nc.vector.tensor_add(qp, qp, sqp[:, 2:6:3, :])
                nrmw = npool.tile([128, 4 * BC], dt.float16, tag="nrmw",
                                  name="nrmw")
                nc.scalar.activation(nrmw[:], qw[:], AF.Sqrt)
                dw = npool.tile([128, 4 * BC], dt.float32, tag="dw",
                                name="dw", bufs=1)
                for g in range(4):
                    nc.vector.tensor_scalar(dw[:, g * BC:(g + 1) * BC],
                                            nrmw[:, g * BC:(g + 1) * BC],
                                            sat[:, g:g + 1], cbt[:, g:g + 1],
                                            OP.mult, OP.add)
                rw = npool.tile([128, 4 * BC], dt.float32, tag="rw",
                                name="rw", bufs=1)
                nc.vector.reciprocal_approx_fast(rw[:], dw[:])
                r16 = npool.tile([128, 4 * BC], dt.float16, tag="r16",
                                 name="r16")
                nc.vector.tensor_copy(r16[:], rw[:])
                nc.vector.tensor_mul(out[:, P(0)], raw[:, P(0)],
                                     r16[:, 0:BC])
                bb = r16[:, BC:3 * BC].rearrange(
                    "p (g u b) -> p g u b", g=2, u=1).broadcast_to(
                    [128, 2, 3, BC])
                nc.vector.tensor_mul(
                    out[:, BC:7 * BC].rearrange(
                        "p (g i b) -> p g i b", g=2, i=3),
                    raw[:, BC:7 * BC].rearrange(
                        "p (g i b) -> p g i b", g=2, i=3), bb)
                nc.vector.tensor_mul(out[:, P(7)], raw[:, P(7)],
                                     r16[:, 3 * BC:])

            def mv_silu(raw, out, at, bt, reg):
                sqw = npool.tile([128, 6 * BC], dt.float16, tag="sqw",
                                 name="sqw")
                sq7 = npool.tile([128, BC], dt.float16, tag="sq7",
                                 name="sq7")
                # blades 1..7 are bias-free so squaring from psum is exact
                nc.scalar.activation(sqw[:, 0:3 * BC], reg[0][:, BC:4 * BC],
                                     AF.Square)
                nc.scalar.activation(sqw[:, 3 * BC:], reg[1][:, 0:3 * BC],
                                     AF.Square)
                nc.scalar.activation(sq7[:], reg[1][:, 3 * BC:], AF.Square)
                q12 = npool.tile([128, 2 * BC], dt.float16, tag="q12",
                                 name="q12")
                sqp = sqw[:].rearrange("p (pl b) -> p pl b", pl=6)
                qp = q12[:].rearrange("p (pl b) -> p pl b", pl=2)
                nc.vector.tensor_add(qp, sqp[:, 0:4:3, :], sqp[:, 1:5:3, :])
                nc.vector.tensor_add(qp, qp, sqp[:, 2:6:3, :])
                invs = [raw[:, P(0)], q12[:, 0:BC], q12[:, BC:], sq7[:]]
                gw = npool.tile([128, 4 * BC], dt.float16, tag="gw",
                                name="gw")
                for g in range(4):
                    nc.scalar.activation(gw[:, g * BC:(g + 1) * BC],
                                         invs[g], AF.Sigmoid,
                                         bias=bt[:, g:g + 1],
                                         scale=at[:, g:g + 1])
                nc.vector.tensor_mul(out[:, P(0)], raw[:, P(0)],
                                     gw[:, 0:BC])
                bb = gw[:, BC:3 * BC].rearrange(
                    "p (g u b) -> p g u b", g=2, u=1).broadcast_to(
                    [128, 2, 3, BC])
                nc.vector.tensor_mul(
                    out[:, BC:7 * BC].rearrange(
                        "p (g i b) -> p g i b", g=2, i=3),
                    raw[:, BC:7 * BC].rearrange(
                        "p (g i b) -> p g i b", g=2, i=3), bb)
                nc.vector.tensor_mul(out[:, P(7)], raw[:, P(7)],
                                     gw[:, 3 * BC:])

            Ht, H2t, HR2t = {}, {}, {}

            # ================= stages 1-2 ===================================
            with tc.tile_pool(name="xacts", bufs=1) as xpool, \
                 tc.tile_pool(name="w_s1", bufs=1) as w1pool:
                Xall = xpool.tile([128, KT_IN * NB * BC], dt.float16,
                                  tag="X", name="X")
                nc.sync.dma_start(Xall[:], xT)
                Xt = {kt: Xall[:, kt * NB * BC:(kt + 1) * NB * BC]
                      for kt in range(KT_IN)}
                lr1t = lin_tile(w1pool, "lr1", lr1w, KT_IN, NIN)
                ll1t = lin_tile(w1pool, "ll1", ll1w, KT_IN, HID)
                w1t_ = w1pool.tile([128, KT_IN * NPATHS * HID], dt.float16,
                                   tag="w1", name="w1")
                nc.sync.dma_start(w1t_[:], w1w)
                # negated copies built on-chip (ScalarE) to keep the
                # critical early DMA path lean
                w1nt_ = w1pool.tile([128, KT_IN * NNEG * HID], dt.float16,
                                    tag="w1nn", name="w1nn")
                for kt in range(KT_IN):
                    for (t0, ln) in NEG_RUNS:
                        sb = (kt * NPATHS + t0) * HID
                        db = (kt * NNEG + NEG_SLOT[t0]) * HID
                        nc.scalar.activation(
                            w1nt_[:, db:db + ln * HID],
                            w1t_[:, sb:sb + ln * HID],
                            AF.Identity, scale=-1.0)
                # stage-3 linear weights: own SBUF (w3pool entered at top),
                # DMA emitted after the stage-1 critical loads
                lrgt = lin_tile(w3pool, "lrg", lrgw, KT_HID, HID)
                llgt = lin_tile(w3pool, "llg", llgw, KT_HID, HID)

                def w1sl(kt, t, mt):
                    base = (kt * NPATHS + t) * HID + mt * 128
                    return w1t_[:, base:base + 128]

                def w1sln(kt, t, mt):
                    base = (kt * NNEG + NEG_SLOT[t]) * HID + mt * 128
                    return w1nt_[:, base:base + 128]

                preload_act(AF.Sqrt)  # load sqrt table during initial DMA

                # --- S1: xr = normalization(lr1(x)) ---
                XRt, regs1 = {}, {}
                for mt in range(MT_IN):
                    regs1[mt] = alloc_reg(f"lr1_{mt}")
                    em = RegionEmitter({0: KT_IN * 2, 1: KT_IN * 2})
                    lin_mms(em, regs1[mt], lr1t, Xt, range(KT_IN), mt)
                    em.done()
                    raw = auxpool.tile([128, NB * BC], dt.float16,
                                       tag="raw", name="raw")
                    evac(regs1[mt], raw)
                    xr = xpool.tile([128, NB * BC], dt.float16,
                                    tag=f"XR_{mt}", name=f"XR_{mt}")
                    normalize(raw, xr, n1sat[mt], n1cbt[mt], regs1[mt])
                    XRt[mt] = xr
                preload_act(AF.Sigmoid, XRt[1])  # for S2 silus

                Qs = {kt: build_q(Xt[kt], XRt[kt]) for kt in range(KT_IN)}
                Qpl = {kt: Qs[kt][:].rearrange("p (pl b) -> p pl b", pl=64)
                       for kt in range(KT_IN)}

                # --- S2: h = silu((ll1(x) + fcgp(x, xr, w1)) / sqrt2) ---
                regs2, ems2 = {}, {}
                tot2 = {b: KT_IN * (2 + NSETS_BANK[b]) for b in (0, 1)}
                for mt in range(3):     # early regions: lin first (X-dep)
                    regs2[mt] = alloc_reg(f"h_{mt}")
                    ems2[mt] = RegionEmitter(tot2)
                    lin_mms(ems2[mt], regs2[mt], ll1t, Xt, range(KT_IN), mt)
                for kt in range(KT_IN):
                    for mt in range(3):
                        gp_mms(ems2[mt], regs2[mt], w1sl, w1sln, Qpl[kt],
                               kt, mt)
                for mt in range(3):
                    ems2[mt].done()
                    raw = auxpool.tile([128, NB * BC], dt.float16,
                                       tag="raw", name="raw")
                    evac(regs2[mt], raw, bias=b1t[mt])
                    h = hpool.tile([128, NB * BC], dt.float16,
                                   tag=f"H_{mt}", name=f"H_{mt}")
                    mv_silu(raw, h, actat[mt], actbt[mt], regs2[mt])
                    Ht[mt] = h
                # mt=3 last (its region slot frees after lr1_0 retires)
                regs2[3] = alloc_reg("h_3")
                em = RegionEmitter(tot2)
                lin_mms(em, regs2[3], ll1t, Xt, range(KT_IN), 3)
                for kt in range(KT_IN):
                    gp_mms(em, regs2[3], w1sl, w1sln, Qpl[kt], kt, 3)
                em.done()
                raw = auxpool.tile([128, NB * BC], dt.float16,
                                   tag="raw", name="raw")
                evac(regs2[3], raw, bias=b1t[3])
                h = hpool.tile([128, NB * BC], dt.float16,
                               tag="H_3", name="H_3")
                mv_silu(raw, h, actat[3], actbt[3], regs2[3])
                Ht[3] = h
                preload_act(AF.Sqrt, Ht[3])  # for S3 normalizes

            # ================= stages 3-6 ===================================
            with tc.tile_pool(name="hracts", bufs=1) as hrpool, \
                 tc.tile_pool(name="w_s5", bufs=1) as w5pool:
                # late weights into the space stage-1 weights vacated
                dt_ = w5pool.tile([128, MT_HID * NPATHS * 128], dt.float16,
                                  tag="dwg", name="dwg")
                nc.sync.dma_start(dt_[:], dwg)
                dn_ = w5pool.tile([128, MT_HID * NNEG * 128], dt.float16,
                                  tag="dwgn", name="dwgn")
                nc.sync.dma_start(dn_[:], dwgn)
                lr2t = lin_tile(w5pool, "lr2", lr2w, KT_HID, HID)
                w2t_ = w5pool.tile([128, KT_HID * NPATHS * NOUT], dt.float16,
                                   tag="w2", name="w2")
                nc.sync.dma_start(w2t_[:], w2w)
                w2nt_ = w5pool.tile([128, KT_HID * NNEG * NOUT], dt.float16,
                                    tag="w2nn", name="w2nn")
                nc.sync.dma_start(w2nt_[:], w2n)
                ll2t = lin_tile(w5pool, "ll2", ll2w, KT_HID, NOUT)

                def dwgsl(ct, t, mt=None):
                    base = (ct * NPATHS + t) * 128
                    return dt_[:, base:base + 128]

                def dwgsln(ct, t, mt=None):
                    base = (ct * NNEG + NEG_SLOT[t]) * 128
                    return dn_[:, base:base + 128]

                def w2sl(kt, t, mt):
                    base = (kt * NPATHS + t) * NOUT + mt * 128
                    return w2t_[:, base:base + 128]

                def w2sln(kt, t, mt):
                    base = (kt * NNEG + NEG_SLOT[t]) * NOUT + mt * 128
                    return w2nt_[:, base:base + 128]

                # --- S3: hr = normalization(lrg(h)) ---
                # early kts for mt 0..2 overlap H_3's silu tail
                HRt, regs3, ems3 = {}, {}, {}
                tot3 = {0: KT_HID * 2, 1: KT_HID * 2}
                for mt in range(3):
                    regs3[mt] = alloc_reg(f"lrg_{mt}")
                    ems3[mt] = RegionEmitter(tot3)
                    lin_mms(ems3[mt], regs3[mt], lrgt, Ht, range(3), mt)
                for mt in range(3):
                    lin_mms(ems3[mt], regs3[mt], lrgt, Ht, [3], mt)
                    ems3[mt].done()
                    raw = auxpool.tile([128, NB * BC], dt.float16,
                                       tag="raw", name="raw")
                    evac(regs3[mt], raw)
                    hr = hrpool.tile([128, NB * BC], dt.float16,
                                     tag=f"HR_{mt}", name=f"HR_{mt}")
                    normalize(raw, hr, ngsat[mt], ngcbt[mt], regs3[mt])
                    HRt[mt] = hr
                regs3[3] = alloc_reg("lrg_3")
                em3 = RegionEmitter(tot3)
                lin_mms(em3, regs3[3], lrgt, Ht, range(KT_HID), 3)
                em3.done()
                raw = auxpool.tile([128, NB * BC], dt.float16,
                                   tag="raw", name="raw")
                evac(regs3[3], raw)
                hr = hrpool.tile([128, NB * BC], dt.float16,
                                 tag="HR_3", name="HR_3")
                normalize(raw, hr, ngsat[3], ngcbt[3], regs3[3])
                HRt[3] = hr
                preload_act(AF.Sigmoid, HRt[3])  # for S4 silus

                # --- S4: h2 = silu((llg(h) + cw_gp(h, hr, wg)) / sqrt2) ---
                regs4, ems4 = {}, {}
                tot4 = {b: KT_HID * 2 + NSETS_BANK[b] for b in (0, 1)}
                for mt in range(3):
                    regs4[mt] = alloc_reg(f"h2_{mt}")
                    ems4[mt] = RegionEmitter(tot4)
                    lin_mms(ems4[mt], regs4[mt], llgt, Ht, range(3), mt)
                for mt in range(4):
                    if mt < 3:
                        lin_mms(ems4[mt], regs4[mt], llgt, Ht, [3], mt)
                    else:
                        regs4[3] = alloc_reg("h2_3")
                        ems4[3] = RegionEmitter(tot4)
                        lin_mms(ems4[3], regs4[3], llgt, Ht, range(KT_HID), 3)
                    q = build_q(Ht[mt], HRt[mt])
                    qpl = q[:].rearrange("p (pl b) -> p pl b", pl=64)
                    gp_mms(ems4[mt], regs4[mt], dwgsl, dwgsln, qpl, mt, None)
                    ems4[mt].done()
                    raw = auxpool.tile([128, NB * BC], dt.float16,
                                       tag="raw", name="raw")
                    evac(regs4[mt], raw, bias=bgt[mt])
                    h2 = hpool.tile([128, NB * BC], dt.float16,
                                    tag=f"H2_{mt}", name=f"H2_{mt}")
                    mv_silu(raw, h2, actat[mt], actbt[mt], regs4[mt])
                    H2t[mt] = h2
                preload_act(AF.Sqrt, H2t[3])  # for S5 normalizes

                # --- S5: hr2 = normalization(lr2(h2)) ---
                regs5, ems5 = {}, {}
                for mt in range(3):
                    regs5[mt] = alloc_reg(f"lr2_{mt}")
                    ems5[mt] = RegionEmitter(tot3)
                    lin_mms(ems5[mt], regs5[mt], lr2t, H2t, range(3), mt)
                for mt in range(3):
                    lin_mms(ems5[mt], regs5[mt], lr2t, H2t, [3], mt)
                    ems5[mt].done()
                    raw = auxpool.tile([128, NB * BC], dt.float16,
                                       tag="raw", name="raw")
                    evac(regs5[mt], raw)
                    hr2 = hpool.tile([128, NB * BC], dt.float16,
                                     tag=f"HR2_{mt}", name=f"HR2_{mt}")
                    normalize(raw, hr2, n2sat[mt], n2cbt[mt], regs5[mt])
                    HR2t[mt] = hr2
                # --- S6 linear first: fills the PE while S5 tails run ---
                reg6 = alloc_reg("out")
                tot6 = {b: KT_HID * (2 + NSETS_BANK[b]) for b in (0, 1)}
                em6 = RegionEmitter(tot6)
                lin_mms(em6, reg6, ll2t, H2t, range(KT_HID), 0)

                regs5[3] = alloc_reg("lr2_3")
                em5 = RegionEmitter(tot3)
                lin_mms(em5, regs5[3], lr2t, H2t, range(KT_HID), 3)
                em5.done()
                raw = auxpool.tile([128, NB * BC], dt.float16,
                                   tag="raw", name="raw")
                evac(regs5[3], raw)
                hr2 = hpool.tile([128, NB * BC], dt.float16,
                                 tag="HR2_3", name="HR2_3")
                normalize(raw, hr2, n2sat[3], n2cbt[3], regs5[3])
                HR2t[3] = hr2

                # --- S6 GP: out = (ll2(h2) + fcgp(h2, hr2, w2)) / sqrt2 ---
                for kt in range(KT_HID):
                    q = build_q(H2t[kt], HR2t[kt])
                    qpl = q[:].rearrange("p (pl b) -> p pl b", pl=64)
                    gp_mms(em6, reg6, w2sl, w2sln, qpl, kt, 0)
                em6.done()
                outs = auxpool.tile([128, NB * BC], dt.float32, tag="outs",
                                    name="outs", bufs=1)
                evac(reg6, outs, bias=b2t[0])
                nc.sync.dma_start(outd, outs[:])

    nc.compile()
    return nc


_PROGRAM = None


def _get_program():
    global _PROGRAM
    if _PROGRAM is None:
        _PROGRAM = build_program()
    return _PROGRAM


def kernel(**inputs):
    from concourse.bass_utils import run_bass_kernel_spmd

    nc = _get_program()
    in_maps = prep_in_maps(inputs)
    res = run_bass_kernel_spmd(nc, in_maps, core_ids=list(range(NCORES)))
    return assemble(res.results)


if __name__ == "__main__":
    nmm = sum(len(TERM_SETS[t]) for t in range(NPATHS))
    print("NEG_TRIPLES:", NEG_TRIPLES)
    print("term-set MMs per (kt,mt):", nmm)
    print("NSETS_BANK:", NSETS_BANK)
